# revision 17
# baseline (speedup 1.0000x reference)
"""Trainium2 Bass kernel for masked multi-head attention (B=8, S=1024, HID=1024, NH=16).

Computation (matches the torch/jax reference):
    q = query @ Wk.T + bk ; k = key @ Wk.T + bk ; v = value @ Wv.T + bv
    per head: scores = q k^T / 8, masked softmax over keys (mask zeroes masked
    positions), out = probs @ v.

Sharding: data-parallel over batch -- batch element b runs on NeuronCore b.

v3 design notes:
  - all matmul operands bf16 (psum f32); final rel err ~6e-3 vs the 2e-2 gate.
  - keys/values host-compacted to unmasked positions (padded to 128).
  - no Q/K bias on device: softmax over k is invariant to per-(q,h) offsets,
    and the surviving per-(k,h) term bk_h.(Wk_h k~) is host-folded into the
    exp bias mb together with the -1e30 pad mask.
  - THE critical constraint is the PE HAM clock gate: any dependency stall in
    the in-order PE queue drops the PE to 4/8 clock for ~30us.  A pure
    dependency-free MM stream sustains 8/8 (measured).  So the attention is a
    lag-one-head software pipeline: while head h's scores stream into psum
    and ScalarE exponentiates them into PT tiles, the PE also runs PV matmuls
    of head h-1 (whose PT tiles finished a full head-step ago) plus Q/K
    projection matmuls of the next head pair as fillers.  Nothing the PE
    issues ever waits on a fresh result.
  - all psum tiles are one bank (N=512 halves): psS scores 3 bufs, psO PV 4
    bufs, psP projections 1 buf = 8 banks.
  - normalize: denominator row per head (ones-column in augmented V), gathered
    via SBUF->SBUF DMA (DVE cannot write odd partition bases), one growing
    reciprocal per pair, selector-matrix matmul broadcasts the pair
    reciprocals to 128 rows; the pair's chain is emitted one head-step late so
    its PE matmuls wait on nothing.
"""

import os
import sys
from contextlib import ExitStack

for _p in ("/opt/trn_rl_repo", "/root/.axon_site/_ro/trn_rl_repo"):
    if os.path.isdir(_p) and _p not in sys.path:
        sys.path.insert(0, _p)

import numpy as np
import ml_dtypes

from concourse import bacc, mybir, tile
from concourse.bass_utils import run_bass_kernel_spmd

B, S, HID, NH = 8, 1024, 1024, 16
HD = HID // NH  # 64
P = 128
JC = HID // P  # 8 contraction chunks for the projections
OB = HID // P  # 8 output-column blocks
NEG = -1.0e30

F32 = mybir.dt.float32
BF16 = mybir.dt.bfloat16
AF = mybir.ActivationFunctionType
BFNP = ml_dtypes.bfloat16

TRACE = os.environ.get("MHA_TRACE", "0") == "1"

_CACHE: dict = {}


def _ensure_axon_ntff_hook():
    """The agent image's antenv lacks axon_hooks; rebuild it from trn_boot's
    ctypes NTFF driver so trace=True can produce per-core profiles."""
    try:
        import antenv.axon_hooks  # noqa: F401

        return
    except ImportError:
        pass
    try:
        import types

        import antenv
        from trn_agent_boot.trn_boot import _ntff_profile_via_ctypes

        m = types.ModuleType("antenv.axon_hooks")
        m._hook = _ntff_profile_via_ctypes("/opt/axon/libaxon_pjrt.so")
        m.get_axon_ntff_profile_hook = lambda: m._hook
        m.set_axon_ntff_profile_hook = lambda h: setattr(m, "_hook", h)
        sys.modules["antenv.axon_hooks"] = m
        antenv.axon_hooks = m
    except Exception as e:  # pragma: no cover
        print(f"ntff hook shim unavailable: {e}", file=sys.stderr)


def _segs(n):
    """Split [0, n) into <=512 pieces aligned to the 512-col psum banks."""
    return [(a, min(a + 512, n)) for a in range(0, n, 512)]


def _build(KB: int):
    """Build the SPMD program for compacted key length KC = KB*128."""
    KC = KB * P
    nc = bacc.Bacc("TRN2", target_bir_lowering=False, debug=False)
    names = {}

    with tile.TileContext(nc) as tc, ExitStack() as ctx:
        dram = ctx.enter_context(tc.tile_pool(name="dram", bufs=1, space="DRAM"))

        def din(nm, shape, dt=F32):
            t = dram.tile(shape, dt, kind="ExternalInput", name=nm, uniquify=False)
            names[nm] = t.name
            return t

        qT_d = din("qT", [HID, S], BF16)
        kT_d = din("kT", [HID, KC], BF16)
        vT_d = din("vT", [HID, KC], BF16)
        WkT_d = din("WkT", [HID, HID], BF16)
        WvT_d = din("WvT", [HID, HID], BF16)
        bvb_d = din("bvb", [P, HID])
        mb_d = din("mb", [P, KB * NH])
        Bsel_d = din("Bsel", [NH, OB * P], BF16)
        outT_d = dram.tile(
            [HID, S], F32, kind="ExternalOutput", name="outT", uniquify=False
        )
        names["out"] = outT_d.name

        res = ctx.enter_context(tc.tile_pool(name="res", bufs=1))
        # input staging (resident for the whole kernel)
        qTt = res.tile([P, JC, S], BF16, tag="qTt")
        kTt = res.tile([P, JC, KC], BF16, tag="kTt")
        vTt = res.tile([P, JC, KC], BF16, tag="vTt")
        WkTt = res.tile([P, JC, HID], BF16, tag="WkTt")
        WvTt = res.tile([P, JC, HID], BF16, tag="WvTt")
        # projected operands
        QT = res.tile([P, OB, S], BF16, tag="QT")        # Q^T  [o, s]
        KT = res.tile([P, OB, KC], BF16, tag="KT")       # K^T  [o, k]
        Vx = res.tile([P, KB, NH * (HD + 1)], BF16, tag="Vx")  # [s(k), head*65]
        bvb = res.tile([P, HID], F32, tag="bvb")
        mb = res.tile([P, KB * NH], F32, tag="mb")
        Bsel = res.tile([NH, OB * P], BF16, tag="Bsel")
        denA = res.tile([NH, S], F32, tag="denA")
        rcA = res.tile([NH, S], F32, tag="rcA")
        rcB = res.tile([NH, S], BF16, tag="rcB")

        # one-bank psum tiles throughout: 3 + 4 + 1 = 8 banks
        psS = ctx.enter_context(tc.tile_pool(name="psS", bufs=3, space="PSUM"))
        psO = ctx.enter_context(tc.tile_pool(name="psO", bufs=4, space="PSUM"))
        psP = ctx.enter_context(tc.tile_pool(name="psP", bufs=1, space="PSUM"))

        ptp = ctx.enter_context(tc.tile_pool(name="ptp", bufs=2 * 2 * KB + 2))
        outp = ctx.enter_context(tc.tile_pool(name="outp", bufs=2))
        onp = ctx.enter_context(tc.tile_pool(name="onp", bufs=2))
        smalls = ctx.enter_context(tc.tile_pool(name="smalls", bufs=3))

        # PE warm-up: dummy matmuls with no data deps run during the initial
        # DMA fill so the HAM clock-gate reaches 8/8 before real work.
        wu = res.tile([P, 512], BF16, tag="wu")
        nc.vector.memset(wu[:], 0.0)
        wu_sink = dram.tile(
            [1, 1], F32, kind="ExternalOutput", name="wu_sink", uniquify=False
        )
        wps = psS.tile([P, 512], F32, tag="S", name="wu_ps")
        NWU = 24
        for i in range(NWU):
            nc.tensor.matmul(
                wps[:], wu[:, 0:P], wu[:], start=(i == 0), stop=(i == NWU - 1)
            )
        wu_sb = res.tile([1, 1], F32, tag="wu_sb")
        nc.vector.tensor_copy(wu_sb[:], wps[0:1, 0:1])
        nc.sync.dma_start(wu_sink[:], wu_sb[:])

        # rcB holds per-head reciprocal rows, filled pairwise as heads finish;
        # zero it so the selector matmul never touches uninitialized NaNs.
        nc.vector.memset(rcB[:], 0.0)
        onef = res.tile([P, 1], F32, tag="onef")
        nc.vector.memset(onef[:], 1.0)
        # last-pair fast path: per-head reciprocal rows + K=1 broadcast lhsT
        rc14f = res.tile([1, S], F32, tag="rc14f")
        rc14b = res.tile([1, S], BF16, tag="rc14b")
        d15 = res.tile([1, S], F32, tag="d15")
        rc15f = res.tile([1, S], F32, tag="rc15f")
        rc15b = res.tile([1, S], BF16, tag="rc15b")
        ones1 = res.tile([1, HD], BF16, tag="ones1")
        nc.vector.tensor_copy(ones1[:], onef[0:1, :].broadcast_to((1, HD)))
        # ones-column of the augmented V (col 64 of each head slot)
        nc.vector.tensor_copy(
            Vx[:].rearrange("p k (h c) -> p k h c", c=HD + 1)[:, :, :, HD],
            onef[:].broadcast_to((P, KB, NH)),
        )

        # ------------- input DMAs, ordered by first use, multi-queue -------
        nc.scalar.dma_start(mb[:], mb_d[:])
        nc.scalar.dma_start(bvb[:], bvb_d[:])
        nc.scalar.dma_start(Bsel[:], Bsel_d[:])
        qs = [nc.sync, nc.gpsimd, nc.scalar]
        for c in range(JC):
            qs[c % 3].dma_start(vTt[:, c, :], vT_d[c * P : (c + 1) * P, :])
            qs[(c + 1) % 3].dma_start(WvTt[:, c, :], WvT_d[c * P : (c + 1) * P, :])
        for c in range(JC):
            qs[c % 3].dma_start(WkTt[:, c, :], WkT_d[c * P : (c + 1) * P, :])
            qs[(c + 1) % 3].dma_start(qTt[:, c, :], qT_d[c * P : (c + 1) * P, :])
            qs[(c + 2) % 3].dma_start(kTt[:, c, :], kT_d[c * P : (c + 1) * P, :])

        # ---------------- phase V: V = value @ Wv^T + bv (natural [s, o]) ---
        for sb in range(KB):
            for a, b in _segs(HID):
                ps = psS.tile([P, 512], F32, tag="S", name=f"psv{sb}_{a}")
                for c in range(JC):
                    nc.tensor.matmul(
                        ps[:, 0 : b - a], vTt[:, c, sb * P : (sb + 1) * P],
                        WvTt[:, c, a:b],
                        start=(c == 0), stop=(c == JC - 1),
                    )
                # evict with +bv into the ones-augmented bf16 layout
                nh0, nh1 = a // HD, b // HD
                nc.vector.tensor_add(
                    Vx[:].rearrange("p k (h c) -> p k h c", c=HD + 1)[
                        :, sb, nh0:nh1, 0:HD
                    ],
                    ps[:, 0 : b - a].rearrange("p (h c) -> p h c", c=HD),
                    bvb[:, a:b].rearrange("p (h c) -> p h c", c=HD),
                )

        # ---------------- projection emitters (Q/K for one ob block) -------
        def qk_proj_steps(ob, pool, tg):
            """Yield (matmul-thunk-or-evict-thunk) steps for ob's Q/K proj."""
            plan = [("q", a, b) for a, b in _segs(S)] + [
                ("k", a, b) for a, b in _segs(KC)
            ]
            for kind, a, b in plan:
                w = b - a
                ps = pool.tile([P, 512], F32, tag=tg, name=f"ps{kind}{ob}_{a}")
                for c in range(JC):
                    src = qTt if kind == "q" else kTt
                    yield lambda ps=ps, c=c, a=a, b=b, w=w, ob=ob, src=src: (
                        nc.tensor.matmul(
                            ps[:, 0:w], WkTt[:, c, ob * P : (ob + 1) * P],
                            src[:, c, a:b],
                            start=(c == 0), stop=(c == JC - 1),
                        )
                    )
                dst = QT if kind == "q" else KT
                yield lambda ps=ps, a=a, b=b, w=w, ob=ob, dst=dst: (
                    nc.vector.tensor_copy(dst[:, ob, a:b], ps[:, 0:w])
                )

        # ob=0 projections run dense before the head pipeline starts, from
        # the deeper psS pool so the psum rotation never stalls the PE
        for step in qk_proj_steps(0, psS, "S"):
            step()

        # ---------------- fused attention pipeline -------------------------
        # head-step h: scores+exp for head h, PV for head h-1, proj fillers
        # for ob=h//2+1, deferred normalize for pair (h-2)//2.
        OpsL: dict = {}
        fillers = None
        pending_norm = []

        def emit_scores(h, kb, a, b):
            g = h // 2
            po = (h % 2) * HD
            Sps = psS.tile([P, 512], F32, tag="S", name=f"S{h}_{kb}_{a}")
            nc.tensor.matmul(
                Sps[:, 0 : b - a],
                KT[po : po + HD, g, kb * P : (kb + 1) * P],
                QT[po : po + HD, g, a:b],
                start=True, stop=True,
            )
            PT = ptp.tile([P, 512], BF16, tag="PT", name=f"PT{h}_{kb}_{a}")
            nc.scalar.activation(
                PT[:, 0 : b - a], Sps[:, 0 : b - a], AF.Exp,
                bias=mb[:, kb * NH + h : kb * NH + h + 1], scale=0.125,
            )
            return PT

        def emit_pv(h, kb, a, b, PT):
            nc.tensor.matmul(
                OpsL[(h, a)][:, 0 : b - a],
                Vx[:, kb, h * (HD + 1) : (h + 1) * (HD + 1)],
                PT[:, 0 : b - a],
                start=(kb == 0), stop=(kb == KB - 1),
            )

        def emit_evict(h):
            """Evict head h's O^T rows + denominator; queue pair normalize."""
            g, po = h // 2, (h % 2) * HD
            OuP = OuPL[g]
            dtmp = smalls.tile([1, S], F32, tag="dtmp", name=f"dtmp{h}")
            for a, b in _segs(S):
                Ops = OpsL.pop((h, a))
                nc.vector.tensor_copy(OuP[po : po + HD, a:b], Ops[0:HD, 0 : b - a])
                nc.vector.tensor_copy(dtmp[0:1, a:b], Ops[HD : HD + 1, 0 : b - a])
            if h == NH - 2:
                # penultimate head: direct per-head reciprocal (no DMA hop,
                # no prefix chain) so the tail never waits on it
                nc.vector.reciprocal_approx_fast(rc14f[:], dtmp[:])
                nc.vector.tensor_copy(rc14b[:], rc14f[:])
                return
            nc.gpsimd.dma_start(denA[h : h + 1, :], dtmp[:])
            if h % 2 == 1:
                ob = g
                nc.vector.reciprocal_approx_fast(
                    rcA[0 : 2 * ob + 2, :], denA[0 : 2 * ob + 2, :]
                )
                nc.vector.tensor_copy(rcB[0 : 2 * ob + 2, :], rcA[0 : 2 * ob + 2, :])
                pending_norm.append(ob)

        def emit_norm(ob):
            """Selector broadcast + normalize + store for head pair ob."""
            OuP = OuPL[ob]
            On = onp.tile([P, S], F32, tag="On", name=f"On{ob}")
            for a, b in _segs(S):
                bc = psS.tile([P, 512], F32, tag="S", name=f"bc{ob}_{a}")
                nc.tensor.matmul(
                    bc[:, 0 : b - a], Bsel[:, ob * P : (ob + 1) * P], rcB[:, a:b],
                    start=True, stop=True,
                )
                nc.vector.tensor_mul(On[:, a:b], OuP[:, a:b], bc[:, 0 : b - a])
            nc.sync.dma_start(outT_d[ob * P : (ob + 1) * P, :], On[:])

        OuPL = {}
        for h in range(NH + 1):
            if h < NH:
                if h % 2 == 0:
                    OuPL[h // 2] = outp.tile(
                        [P, S], F32, tag="OuP", name=f"OuP{h // 2}"
                    )
                if h % 2 == 0 and h // 2 + 1 < OB:
                    fillers = qk_proj_steps(h // 2 + 1, psP, "P")
                PTprev = PTcur if h > 0 else None
                PTcur = []
                slot = 0
                for kb in range(KB):
                    for a, b in _segs(S):
                        PTcur.append((kb, a, b, emit_scores(h, kb, a, b)))
                        if h > 0:
                            kb2, a2, b2, PT2 = PTprev[slot]
                            if kb2 == 0:
                                OpsL[(h - 1, a2)] = psO.tile(
                                    [HD + 1, 512], F32, tag="O",
                                    name=f"O{h - 1}_{a2}",
                                )
                            emit_pv(h - 1, kb2, a2, b2, PT2)
                        if fillers is not None:
                            for _ in range(2):
                                nxt = next(fillers, None)
                                if nxt is None:
                                    fillers = None
                                    break
                                nxt()
                        slot += 1
                        # flush the deferred normalize near the step's end
                        if slot == 2 * KB - 1 and pending_norm:
                            emit_norm(pending_norm.pop(0))
                if h > 0:
                    emit_evict(h - 1)
            else:
                # tail: last head half-major so each half's DVE chain overlaps
                # the other half's PE work; last pair uses the direct
                # reciprocal path (no DMA hop / prefix dependency)
                h15 = NH - 1
                OuP7 = OuPL[OB - 1]
                byhalf: dict = {}
                for kb, a, b, PT in PTcur:
                    byhalf.setdefault((a, b), []).append((kb, PT))

                def tail_half(a, b):
                    w = b - a
                    OpsL[(h15, a)] = psO.tile(
                        [HD + 1, 512], F32, tag="O", name=f"O{h15}_{a}"
                    )
                    for kb, PT in byhalf[(a, b)]:
                        emit_pv(h15, kb, a, b, PT)
                    Ops = OpsL.pop((h15, a))
                    nc.vector.tensor_copy(OuP7[HD:P, a:b], Ops[0:HD, 0:w])
                    nc.vector.tensor_copy(d15[0:1, a:b], Ops[HD : HD + 1, 0:w])
                    nc.vector.reciprocal_approx_fast(
                        rc15f[0:1, a:b], d15[0:1, a:b]
                    )
                    nc.vector.tensor_copy(rc15b[0:1, a:b], rc15f[0:1, a:b])

                (a0, b0), (a1, b1) = sorted(byhalf)
                tail_half(a0, b0)
                while pending_norm:  # pair 6, fills the PE during the chain
                    emit_norm(pending_norm.pop(0))
                tail_half(a1, b1)
                On7 = onp.tile([P, S], F32, tag="On", name="On7")
                for a, b in _segs(S):
                    w = b - a
                    bc = psS.tile([P, 512], F32, tag="S", name=f"bc7_{a}")
                    nc.tensor.matmul(
                        bc[0:HD, 0:w], ones1[:], rc14b[0:1, a:b],
                        start=True, stop=True,
                    )
                    nc.tensor.matmul(
                        bc[HD:P, 0:w], ones1[:], rc15b[0:1, a:b],
                        start=True, stop=True,
                    )
                    nc.vector.tensor_mul(On7[:, a:b], OuP7[:, a:b], bc[:, 0:w])
                    nc.sync.dma_start(
                        outT_d[(OB - 1) * P : OB * P, a:b], On7[:, a:b]
                    )

    nc.compile()
    return nc, names


def _prep(query, key, value, attention_mask, Wk, bk, Wv, bv):
    """Host-side sharding + layout prep. Returns (KB, in_maps, empty_batches)."""
    query = np.asarray(query, dtype=np.float32)
    key = np.asarray(key, dtype=np.float32)
    value = np.asarray(value, dtype=np.float32)
    mask = np.asarray(attention_mask).reshape(B, S) != 0
    Wk = np.asarray(Wk, dtype=np.float32)
    bk = np.asarray(bk, dtype=np.float32)
    Wv = np.asarray(Wv, dtype=np.float32)
    bv = np.asarray(bv, dtype=np.float32)

    idxs, counts = [], []
    for b in range(B):
        ix = np.flatnonzero(mask[b])
        idxs.append(ix)
        counts.append(len(ix))
    KC = max(int(np.ceil(max(max(counts), 1) / P)) * P, P)
    KB = KC // P

    WkT = np.ascontiguousarray(Wk.T.astype(BFNP))
    WvT = np.ascontiguousarray(Wv.T.astype(BFNP))
    bvb = np.ascontiguousarray(np.broadcast_to(bv, (P, HID))).astype(np.float32)
    # per-head k-side score bias vectors: WH[h] = Wk[head h rows].T @ bk[head h]
    WH = np.einsum(
        "hdo,hd->ho", Wk.reshape(NH, HD, HID), bk.reshape(NH, HD)
    )  # [NH, HID]
    # head-pair selector for the reciprocal broadcast matmul
    Bsel = np.zeros((NH, OB * P), dtype=BFNP)
    for ob in range(OB):
        Bsel[2 * ob, ob * P : ob * P + HD] = 1
        Bsel[2 * ob + 1, ob * P + HD : (ob + 1) * P] = 1

    in_maps = []
    empty = []
    for b in range(B):
        n = counts[b]
        if n == 0:
            empty.append(b)
        ix = idxs[b] if n > 0 else np.array([0])
        pad = np.concatenate([ix, np.full(KC - len(ix), ix[0], dtype=ix.dtype)])
        key_c = key[b][pad]  # [KC, HID] compacted keys (f32)
        # exp bias: pad mask plus the per-(k,h) bias term, 0.125-scaled
        C = key_c @ WH.T  # [KC, NH]
        mbm = 0.125 * C
        mbm[n:, :] = NEG
        mb = np.ascontiguousarray(
            mbm.reshape(KB, P, NH).transpose(1, 0, 2).reshape(P, KB * NH)
        ).astype(np.float32)
        qT = np.ascontiguousarray(query[b].astype(BFNP).T)
        kT = np.ascontiguousarray(key_c.astype(BFNP).T)
        vT = np.ascontiguousarray(value[b][pad].astype(BFNP).T)
        in_maps.append(
            {
                "qT": qT,
                "kT": kT,
                "vT": vT,
                "WkT": WkT,
                "WvT": WvT,
                "bvb": bvb,
                "mb": mb,
                "Bsel": Bsel,
            }
        )
    return KB, in_maps, empty


def kernel(key, value, query, attention_mask, Wk, bk, Wv, bv):
    KB, in_maps, empty = _prep(query, key, value, attention_mask, Wk, bk, Wv, bv)

    if KB not in _CACHE:
        _CACHE[KB] = _build(KB)
    nc, names = _CACHE[KB]

    # remap host arrays onto the (possibly uniquified) dram tensor names
    mapped = [{names[k]: v for k, v in m.items()} for m in in_maps]
    if TRACE:
        _ensure_axon_ntff_hook()
    res = run_bass_kernel_spmd(nc, mapped, list(range(B)), trace=TRACE)
    if TRACE and res.exec_time_ns is not None:
        print(f"HW exec time: {res.exec_time_ns} ns")

    out = np.empty((B, S, HID), dtype=np.float32)
    for b in range(B):
        out[b] = res.results[b][names["out"]].T
    for b in empty:
        out[b] = 0.0
    return out
